# revision 14
# baseline (speedup 1.0000x reference)
"""Trainium2 Bass kernel for MIGAttention (topk token masking + GQA attention).

Shapes (hardcoded): B=4, N=2048, C=1024, H=16 heads, HKV=4 kv-heads, DH=64,
keep-ratio 0.7 -> k = 1433 selected tokens per batch row.

Sharding: 8 cores = (batch b in 0..3) x (query-half h in 0..1).  Each core
receives x[b].T with token columns rolled by h*1024 so that its own query
half always occupies columns 0..1023 -> a single SPMD program for all cores.
Each core computes the full gate+topk mask and K/V for all 2048 tokens of its
batch, and attention + output projection for its 1024 queries.
"""

import contextlib
import sys

import numpy as np

if "/opt/trn_rl_repo" not in sys.path:
    sys.path.insert(0, "/opt/trn_rl_repo")

import concourse.bass as bass  # noqa: F401
import concourse.bass_isa as bass_isa
import concourse.mybir as mybir
from concourse import bacc
from concourse.tile import TileContext

F32 = mybir.dt.float32
F32R = mybir.dt.float32r
BF16 = mybir.dt.bfloat16
I32 = mybir.dt.int32
AF = mybir.ActivationFunctionType
ALU = mybir.AluOpType

B, N, C = 4, 2048, 1024
H, HKV, DH = 16, 4, 64
NQ = N // 2          # queries per core
KSEL = 1433          # max(1, int(N * 0.7))
CC = C // 128        # contraction chunks (8)
KC = N // 128        # key chunks (16)
QT_D = H * DH        # 1024
KV_D = HKV * DH      # 256
VROW = KV_D + HKV    # 260: per kv-head 64 dims + a ones column (65 each)
N_ROUNDS = 6         # topk threshold refinement rounds
LO0, W0 = -8.0, 16.0  # initial logit search interval (logit std ~0.65)


def r32(ap):
    return ap.bitcast(F32R)


def _emit(nc, tc, ctx, io):
    xT, wq, wk, wv, rw, wo, out_d = (
        io["xT"], io["wq"], io["wk"], io["wv"], io["rw"], io["wo"], io["out"])

    # ---------------- long-lived pools ----------------
    const = ctx.enter_context(tc.tile_pool(name="const", bufs=1))
    small = ctx.enter_context(tc.tile_pool(name="small", bufs=1))
    big = ctx.enter_context(tc.tile_pool(name="big", bufs=1))
    dram = ctx.enter_context(tc.tile_pool(name="dram", bufs=1, space="DRAM"))

    # phase-scoped stacks (closed as soon as their tiles die); pools are
    # created lazily right before first use (space is reserved at creation)
    px_ctx = contextlib.ExitStack()   # xT (alive through all projections)
    pa_ctx = contextlib.ExitStack()   # router/refinement scratch
    pq_ctx = contextlib.ExitStack()   # wq (Q projection runs first)
    pm_ctx = contextlib.ExitStack()   # m_rep
    pkv_ctx = contextlib.ExitStack()  # wk, wv
    px = px_ctx.enter_context(tc.tile_pool(name="px", bufs=1))
    psum1 = px_ctx.enter_context(tc.tile_pool(name="psum1", bufs=3, space="PSUM"))
    pm = pm_ctx.enter_context(tc.tile_pool(name="pm", bufs=1))
    pa = pa_ctx.enter_context(tc.tile_pool(name="pa", bufs=1))
    psum_r = pa_ctx.enter_context(tc.tile_pool(name="psum_r", bufs=1, space="PSUM"))

    # ---------------- constants ----------------
    ones_row = const.tile([1, 128], F32)
    nc.vector.memset(ones_row, 1.0)
    iota128_i = const.tile([128, 1], I32)
    nc.gpsimd.iota(iota128_i, pattern=[[0, 1]], base=1, channel_multiplier=1)
    iota128 = const.tile([128, 1], F32)
    nc.vector.tensor_copy(iota128, iota128_i)

    # ---------------- loads (phase 1) ----------------
    xT_sb = px.tile([128, CC, N], F32R)
    for cc in range(CC):
        nc.sync.dma_start(xT_sb[:, cc, :],
                          xT[cc * 128:(cc + 1) * 128, :].bitcast(F32R))
    rw_sb = pa.tile([128, CC], F32)
    for cc in range(CC):
        sl = slice(cc * 128, (cc + 1) * 128)
        nc.sync.dma_start(rw_sb[:, cc:cc + 1], rw[sl, :])

    # ---------------- router: logits = x @ rw  (full fp32 for exact order;
    # streams its own fp32 copy of x since xT_sb is fp32r-typed)
    logits_sb = pa.tile([1, N], F32)
    xr_pool = pa_ctx.enter_context(tc.tile_pool(name="xr_pool", bufs=2))
    rps = [psum_r.tile([1, 512], F32, tag=f"router_ps{g}", name=f"router_ps{g}")
           for g in range(4)]
    for cc in range(CC):
        xr = xr_pool.tile([128, N], F32, tag="xr", name=f"xr{cc}")
        nc.sync.dma_start(xr, xT[cc * 128:(cc + 1) * 128, :])
        for g in range(4):
            nc.tensor.matmul(
                rps[g], rw_sb[:, cc:cc + 1], xr[:, g * 512:(g + 1) * 512],
                start=(cc == 0), stop=(cc == CC - 1))
    for g in range(4):
        nc.vector.tensor_copy(logits_sb[:, g * 512:(g + 1) * 512], rps[g])

    # replicate logits across all 128 partitions (K=1 matmul broadcast)
    lrep = pa.tile([128, N], F32)
    for g in range(4):
        ps = psum_r.tile([128, 512], F32, tag="bcast_ps")
        nc.tensor.matmul(ps, ones_row, logits_sb[:, g * 512:(g + 1) * 512],
                         start=True, stop=True)
        nc.vector.tensor_copy(lrep[:, g * 512:(g + 1) * 512], ps)

    # ---------------- topk threshold refinement ----------------
    # invariant: v* (the KSEL-th largest logit) is in (lo, lo + w]
    lo = small.tile([128, 1], F32)
    nc.vector.memset(lo, LO0)
    neg_edges = small.tile([128, 1], F32)
    acc = small.tile([128, 1], F32)
    sel = small.tile([128, 1], F32)
    ssum = small.tile([128, 1], F32)
    sign_scr = pa.tile([128, N], BF16)  # Sign output is never read
    thr_acc = float(2 * KSEL - N)  # acc = #gt - #lt ; acc>=thr <=> #gt>=KSEL
    for r in range(N_ROUNDS):
        wstep = W0 / (128.0 ** (r + 1))
        # neg_edges[p] = -((p+1)*wstep + lo)  computed as iota*(-wstep) - lo
        nc.vector.scalar_tensor_tensor(
            neg_edges, iota128, -wstep, lo, op0=ALU.mult, op1=ALU.subtract)
        nc.scalar.activation(sign_scr, lrep, AF.Sign, bias=neg_edges,
                             scale=1.0, accum_out=acc)
        nc.vector.tensor_single_scalar(sel, acc, thr_acc, op=ALU.is_ge)
        nc.gpsimd.partition_all_reduce(ssum, sel, channels=128,
                                       reduce_op=bass_isa.ReduceOp.add)
        # lo += ssum * wstep   (bit-identical to the edge it selects)
        nc.vector.scalar_tensor_tensor(
            lo, ssum, wstep, lo, op0=ALU.mult, op1=ALU.add)

    # m = (logit > lo) * sigmoid(logit)   per token, replicated on partitions
    grep = pa.tile([128, N], F32)
    nc.scalar.activation(grep, lrep, AF.Sigmoid)
    m_rep = pm.tile([128, N], F32)
    nc.vector.scalar_tensor_tensor(
        m_rep, lrep, lo, grep, op0=ALU.is_gt, op1=ALU.mult)

    # m in token-major layout for V row scaling: m_v[p, i] = m[i*128 + p]
    m_dram = dram.tile([N], F32)
    nc.sync.dma_start(m_dram, m_rep[0:1, :])
    m_v = small.tile([128, KC], F32)
    nc.sync.dma_start(m_v, m_dram.rearrange("(i p) -> p i", p=128))
    pa_ctx.close()

    # ---------------- projections: QT first (then wq freed) ----------------
    # QT[d, q] for my 1024 queries (columns 0..1023 of the rolled xT).
    # Slot layout is permuted so each q-head lands on the same partition range
    # as its GQA kv-head in KT: head h -> slot (h%4)+4*(h//8), partition base
    # ((h//4)%2)*64.  Slot j therefore holds heads (ha, ha+4), ha = j if j<4
    # else j+4, and wq columns are picked per head via a stride-4 head view.
    pq = pq_ctx.enter_context(tc.tile_pool(name="pq", bufs=1))
    wq_sb = pq.tile([128, CC, QT_D], F32R)
    for cc in range(CC):
        nc.sync.dma_start(wq_sb[:, cc, :],
                          wq[cc * 128:(cc + 1) * 128, :].bitcast(F32R))
    qt_sb = big.tile([128, H // 2, NQ], F32R)
    for j in range(H // 2):
        for g in range(NQ // 512):
            ps = psum1.tile([128, 512], F32, tag="proj_ps",
                            name=f"q_ps{j}_{g}")
            qs = slice(g * 512, (g + 1) * 512)
            for cc in range(CC):
                nc.tensor.matmul(
                    ps, wq_sb[:, cc, j * 128:(j + 1) * 128],
                    xT_sb[:, cc, qs],
                    start=(cc == 0), stop=(cc == CC - 1))
            nc.vector.tensor_tensor(qt_sb[:, j, qs], ps, m_rep[:, qs], op=ALU.mult)
    pq_ctx.close()

    # ---------------- KT, V ----------------
    pkv = pkv_ctx.enter_context(tc.tile_pool(name="pkv", bufs=1))
    wk_sb = pkv.tile([128, CC, KV_D], F32R)
    wv_sb = pkv.tile([128, CC, KV_D], F32R)
    for cc in range(CC):
        sl = slice(cc * 128, (cc + 1) * 128)
        nc.sync.dma_start(wk_sb[:, cc, :], wk[sl, :].bitcast(F32R))
        nc.sync.dma_start(wv_sb[:, cc, :], wv[sl, :].bitcast(F32R))

    # KT[d, t] for all 2048 tokens
    kt_sb = big.tile([128, 2, N], F32R)
    for j in range(2):
        for g in range(N // 512):
            ps = psum1.tile([128, 512], F32, tag="proj_ps",
                            name=f"k_ps{j}_{g}")
            ts_ = slice(g * 512, (g + 1) * 512)
            for cc in range(CC):
                nc.tensor.matmul(
                    ps, wk_sb[:, cc, j * 128:(j + 1) * 128],
                    xT_sb[:, cc, ts_],
                    start=(cc == 0), stop=(cc == CC - 1))
            nc.vector.tensor_tensor(kt_sb[:, j, ts_], ps, m_rep[:, ts_], op=ALU.mult)

    # V[t, d] (bf16), stored per kv-head as 65 columns: 64 dims + ones col
    v_sb = big.tile([128, KC, VROW], BF16)
    for hk in range(HKV):
        nc.vector.memset(v_sb[:, :, hk * 65 + 64:hk * 65 + 65], 1.0)
    v_dst_view = v_sb.rearrange("p i (h e) -> p i h e", e=65)
    for i in range(KC):
        ps = psum1.tile([128, 512], F32, tag="proj_ps", name=f"v_ps{i}")
        pv = ps[:, 0:KV_D]
        for cc in range(CC):
            nc.tensor.matmul(
                pv, xT_sb[:, cc, i * 128:(i + 1) * 128],
                wv_sb[:, cc, :],
                start=(cc == 0), stop=(cc == CC - 1))
        nc.vector.tensor_scalar(
            v_dst_view[:, i, :, 0:64],
            pv.rearrange("p (h e) -> p h e", e=64),
            m_v[:, i:i + 1], None, op0=ALU.mult)
    pkv_ctx.close()
    pm_ctx.close()
    px_ctx.close()  # free xT + phase-1 PSUM

    # ---------------- phase 2: attention ----------------
    ph2_ctx = contextlib.ExitStack()
    ph2 = ph2_ctx.enter_context(tc.tile_pool(name="ph2", bufs=1))
    wo_sb = ph2.tile([128, CC, C], F32R)
    for cc in range(CC):
        nc.sync.dma_start(wo_sb[:, cc, :],
                          wo[cc * 128:(cc + 1) * 128, :].bitcast(F32R))

    patt_ctx = contextlib.ExitStack()
    p_pool = patt_ctx.enter_context(tc.tile_pool(name="p_pool", bufs=2))
    lg_pool = patt_ctx.enter_context(
        tc.tile_pool(name="lg_pool", bufs=2, space="PSUM"))
    att_pool = patt_ctx.enter_context(
        tc.tile_pool(name="att_pool", bufs=1, space="PSUM"))
    oT_sb = ph2.tile([128, CC, NQ], F32R)
    recip = ph2.tile([1, NQ], F32)
    rrep = ph2.tile([64, NQ], F32)
    odd_scr = ph2.tile([64, NQ], F32R)

    inv_sqrt_dh = float(1.0 / np.sqrt(DH))
    KQ = KC // 4  # key chunks per P buffer
    # Head pairs (ha, ha+4) sit on disjoint partition halves (row-packed PE).
    pair_heads = [(ha, ha + 4) for ha in (0, 1, 2, 3, 8, 9, 10, 11)]
    for hp, pair in enumerate(pair_heads):
        att_ps = [att_pool.tile([65, NQ], F32, tag=f"att{m}", name=f"att{hp}_{m}")
                  for m in range(2)]
        pend = []  # pipelined attv matmuls: emitted one kc behind logits/exp
        for quarter in range(KC // KQ):
            p_t = p_pool.tile([128, KQ, N], BF16, tag="p_t", name=f"p_{hp}_{quarter}")
            for kci in range(KQ):
                kc = quarter * KQ + kci
                lg = [lg_pool.tile([128, NQ], F32, tag="lg",
                                   name=f"lg{hp}_{kc}_{m2}") for m2 in range(2)]
                for m in range(2):
                    h = pair[m]
                    base = ((h // 4) % 2) * 64
                    rs = slice(base, base + 64)
                    jq = (h % 4) + 4 * (h // 8)
                    for g in range(NQ // 512):
                        nc.tensor.matmul(
                            lg[m][:, g * 512:(g + 1) * 512],
                            kt_sb[rs, h // 8, kc * 128:(kc + 1) * 128],
                            qt_sb[rs, jq, g * 512:(g + 1) * 512],
                            start=True, stop=True)
                for m in range(2):
                    nc.scalar.activation(
                        p_t[:, kci, m * NQ:(m + 1) * NQ], lg[m], AF.Exp,
                        scale=inv_sqrt_dh)
                # emit previous kc's attv matmuls now (keeps PE streaming)
                for f in pend:
                    f()
                pend = []

                def attv(p_t=p_t, kci=kci, kc=kc):
                    for m in range(2):
                        hk = pair[m] // 4
                        for g in range(NQ // 512):
                            nc.tensor.matmul(
                                att_ps[m][:, g * 512:(g + 1) * 512],
                                v_sb[:, kc, hk * 65:hk * 65 + 65],
                                p_t[:, kci,
                                    m * NQ + g * 512:m * NQ + (g + 1) * 512],
                                start=(kc == 0), stop=(kc == KC - 1))

                pend.append(attv)
        for f in pend:
            f()
        # evict: oT[h] = att[0:64] / denom(att[64])
        for m in range(2):
            h = pair[m]
            nc.vector.reciprocal(recip, att_ps[m][64:65, :])
            nc.gpsimd.partition_broadcast(rrep, recip, channels=64)
            if h % 2 == 0:
                nc.vector.tensor_tensor(
                    oT_sb[0:64, h // 2, :], att_ps[m][0:64, :], rrep,
                    op=ALU.mult)
            else:
                # DVE can't shift partition base 0 -> 64; go via scratch + DMA
                nc.vector.tensor_tensor(
                    odd_scr, att_ps[m][0:64, :], rrep, op=ALU.mult)
                nc.sync.dma_start(oT_sb[64:128, h // 2, :], odd_scr)

    patt_ctx.close()
    # ---------------- phase 3: output projection ----------------
    ph3_ctx = contextlib.ExitStack()
    psum3 = ph3_ctx.enter_context(tc.tile_pool(name="psum3", bufs=4, space="PSUM"))
    out_pool = ph3_ctx.enter_context(tc.tile_pool(name="out_pool", bufs=2))
    for tt in range(NQ // 128):
        out_sb = out_pool.tile([128, C], F32, tag="out_sb", name=f"out_sb{tt}")
        for og in range(C // 512):
            ps = psum3.tile([128, 512], F32, tag="out_ps", name=f"out_ps{tt}_{og}")
            for dd in range(CC):
                nc.tensor.matmul(
                    ps, oT_sb[:, dd, tt * 128:(tt + 1) * 128],
                    wo_sb[:, dd, og * 512:(og + 1) * 512],
                    start=(dd == 0), stop=(dd == CC - 1))
            nc.scalar.copy(out_sb[:, og * 512:(og + 1) * 512], ps)
        nc.sync.dma_start(out_d[tt * 128:(tt + 1) * 128, :], out_sb)
    ph3_ctx.close()
    ph2_ctx.close()


_NC = None


def build_program():
    global _NC
    if _NC is not None:
        return _NC
    from contextlib import ExitStack

    nc = bacc.Bacc("TRN2", target_bir_lowering=False, debug=False, num_devices=8)
    io = {
        "xT": nc.dram_tensor("xT", (C, N), F32, kind="ExternalInput").ap(),
        "wq": nc.dram_tensor("wq", (C, QT_D), F32, kind="ExternalInput").ap(),
        "wk": nc.dram_tensor("wk", (C, KV_D), F32, kind="ExternalInput").ap(),
        "wv": nc.dram_tensor("wv", (C, KV_D), F32, kind="ExternalInput").ap(),
        "rw": nc.dram_tensor("rw", (C, 1), F32, kind="ExternalInput").ap(),
        "wo": nc.dram_tensor("wo", (C, C), F32, kind="ExternalInput").ap(),
        "out": nc.dram_tensor("out", (NQ, C), F32, kind="ExternalOutput").ap(),
    }
    with TileContext(nc) as tc:
        with ExitStack() as ctx:
            _emit(nc, tc, ctx, io)
    nc.compile()
    _NC = nc
    return nc


def _permute_wq(wq):
    """Column-permute wq so QT slot j's 128 cols = heads (ha, ha+4) contig."""
    wq = np.asarray(wq, np.float32).reshape(C, H, DH)
    order = []
    for j in range(H // 2):
        ha = j if j < 4 else j + 4
        order += [ha, ha + 4]
    return np.ascontiguousarray(wq[:, order, :].reshape(C, H * DH))


def make_in_maps(x, router_w, wq, wk, wv, wo):
    wq = _permute_wq(wq)
    in_maps = []
    for core in range(8):
        b, h = core // 2, core % 2
        xT_core = np.ascontiguousarray(
            np.roll(np.asarray(x[b], np.float32).T, -h * NQ, axis=1))
        in_maps.append({
            "xT": xT_core,
            "wq": np.ascontiguousarray(wq, dtype=np.float32),
            "wk": np.ascontiguousarray(wk, dtype=np.float32),
            "wv": np.ascontiguousarray(wv, dtype=np.float32),
            "rw": np.ascontiguousarray(router_w, dtype=np.float32),
            "wo": np.ascontiguousarray(wo, dtype=np.float32),
        })
    return in_maps


def _numpy_fallback(x, router_w, router_b, wq, bq, wk, bk, wv, bv, wo, bo):
    x = np.asarray(x, np.float32)
    gate = 1.0 / (1.0 + np.exp(-(x @ router_w + router_b)))
    xg = x * gate
    scores = gate[..., 0]
    idx = np.argsort(-scores, axis=-1, kind="stable")[:, :KSEL]
    mask = np.zeros((x.shape[0], x.shape[1]), np.float32)
    np.put_along_axis(mask, idx, 1.0, axis=1)
    xg = xg * mask[..., None]
    q = (xg @ wq + bq).reshape(B, N, H, DH)
    kk = np.repeat((xg @ wk + bk).reshape(B, N, HKV, DH), H // HKV, axis=2)
    v = np.repeat((xg @ wv + bv).reshape(B, N, HKV, DH), H // HKV, axis=2)
    att = np.einsum("bqhd,bkhd->bhqk", q, kk) / np.float32(np.sqrt(DH))
    att = att - att.max(-1, keepdims=True)
    att = np.exp(att)
    att = att / att.sum(-1, keepdims=True)
    o = np.einsum("bhqk,bkhd->bqhd", att, v).reshape(B, N, C)
    return (o @ wo + bo).astype(np.float32)


def kernel(x, router_w, router_b, wq, bq, wk, bk, wv, bv, wo, bo):
    x = np.asarray(x)
    biases = [router_b, bq, bk, bv, bo]
    if any(float(np.abs(np.asarray(t)).max()) != 0.0 for t in biases):
        # The device program folds away the (identically zero) biases; fall
        # back to an exact host implementation if that assumption breaks.
        return _numpy_fallback(x, router_w, router_b, wq, bq, wk, bk, wv, bv,
                               wo, bo)

    from concourse import bass_utils

    nc = build_program()
    in_maps = make_in_maps(x, router_w, wq, wk, wv, wo)
    res = bass_utils.run_bass_kernel_spmd(nc, in_maps, core_ids=list(range(8)))
    out = np.empty((B, N, C), np.float32)
    for core in range(8):
        b, h = core // 2, core % 2
        out[b, h * NQ:(h + 1) * NQ, :] = res.results[core]["out"]
    return out
